# revision 46
# baseline (speedup 1.0000x reference)
"""Distributed Trainium2 kernel for the linear-attention transformer block.

Math (per batch element b):
  Q = elu(x @ Wq), K = elu(x @ Wk), V = x @ Wv   (per-head d=64)
  KV_h = K_h^T V_h  [64,64];  Ksum_h = sum_n K_h[n]  [64]
  attn_h = (Q_h @ KV_h) / (Q_h . Ksum_h)
  out = LayerNorm(x + attn @ Wo) * gamma + beta

Sharding: 16384 tokens over 8 cores (2048 each; core c owns batch c//2,
half c%2). Each core computes Q/K/V only for its tokens, partial KV/Ksum,
then a 266KB AllReduce over core pairs {2b, 2b+1} completes the KV stats;
attention + output projection + LayerNorm finish locally.

Precision: projections run in f32r (full-rate on the PE for moving dims
>=256, ~17-bit effective mantissa), which keeps the 1/(Q.Ksum)
denominators accurate without multi-term bf16 splits. KV/attention
numerator uses bf16 where it only feeds the (residual-dominated) output.
ELU is computed as max(exp(min(x,0)) - 1, x) - one op per engine stage
(Pool min, ACT exp, DVE fused scalar_tensor_tensor).
"""

import sys

sys.path.insert(0, "/opt/trn_rl_repo")

import numpy as np
import ml_dtypes

import concourse.bass as bass
import concourse.mybir as mybir
import concourse.tile as tile
from concourse import bacc
from concourse.bass_utils import run_bass_kernel_spmd

AF = mybir.ActivationFunctionType
OP = mybir.AluOpType
F32 = mybir.dt.float32
F32R = mybir.dt.float32r
BF16 = mybir.dt.bfloat16
FP8 = mybir.dt.float8e4   # IEEE e4m3: max finite 240 (NOT the fn variant)
DR = mybir.MatmulPerfMode.DoubleRow
QC_DESCALE = 2.0 ** -21   # x2q(2^11)@wq1(2^10) == x1q(2^5)@wq2(2^16)
V_DESCALE = 2.0 ** -15    # x1q(2^5) @ wv8(2^10)

B, N, D = 4, 4096, 1024
H, HD = 16, 64
TOK = 2048            # tokens per core
NCORES = 8
LN_EPS = 1e-3
P = 128
KC = D // P           # 8 contraction chunks
TC = TOK // P         # 16 token chunks of 128
TQ = TOK // 512       # 4 token chunks of 512

WARMUP_MM = 64
Q_CORR = True     # fp8 DoubleRow correction terms on the Q projection

LAST_RESULT = None    # BassKernelResults of the most recent run (for test.py)


def _build(apply_bias, apply_gamma, apply_beta):
    nc = bacc.Bacc("TRN2", target_bir_lowering=False, debug=False, num_devices=NCORES)

    xt = nc.dram_tensor("xt", [D, TOK], F32R, kind="ExternalInput")
    wq_d = nc.dram_tensor("wq", [D, D], F32R, kind="ExternalInput")
    wk_d = nc.dram_tensor("wk", [D, D], F32R, kind="ExternalInput")
    # fp8 operands: x1q = x*2^5, x2q = (x - rn11(x))*2^11, wq1 = Wq*2^10,
    # wq2 = (Wq - rn11(Wq))*2^16 (all e4m3, host-quantized).
    x1q_d = nc.dram_tensor("x1q", [D, TOK], FP8, kind="ExternalInput")
    x2q_d = nc.dram_tensor("x2q", [D, TOK], FP8, kind="ExternalInput")
    wq1_d = nc.dram_tensor("wq1", [D, D], FP8, kind="ExternalInput")
    wq2_d = nc.dram_tensor("wq2", [D, D], FP8, kind="ExternalInput")
    wv_d = nc.dram_tensor("wv", [D, D], F32R, kind="ExternalInput")
    wo_d = nc.dram_tensor("wo", [D, D], BF16, kind="ExternalInput")
    xres = nc.dram_tensor("xres", [TOK, D], F32, kind="ExternalInput")
    e_sel = nc.dram_tensor("e_sel", [2, P], BF16, kind="ExternalInput")
    if apply_bias:
        bq_d = nc.dram_tensor("bq", [D], F32, kind="ExternalInput")
        bk_d = nc.dram_tensor("bk", [D], F32, kind="ExternalInput")
        bv_d = nc.dram_tensor("bv", [D], F32, kind="ExternalInput")
        bo_d = nc.dram_tensor("bo", [D], F32, kind="ExternalInput")
    if apply_gamma:
        gamma_d = nc.dram_tensor("gamma", [D], F32, kind="ExternalInput")
    if apply_beta:
        beta_d = nc.dram_tensor("beta", [D], F32, kind="ExternalInput")
    out_d = nc.dram_tensor("out", [TOK, D], F32, kind="ExternalOutput")

    r8 = lambda t: t.ap().rearrange("(ko p) n -> p ko n", p=P)

    def bcast_row(dram_vec, sb_tile):
        # DMA-broadcast a [D] vector to [P, D] (stride-0 partition dim).
        src = bass.AP(
            tensor=dram_vec.ap().tensor,
            offset=dram_vec.ap().offset,
            ap=[[0, P]] + list(dram_vec.ap().ap),
        )
        nc.sync.dma_start(out=sb_tile, in_=src)

    with tile.TileContext(nc) as tc:
        with (
            tc.tile_pool(name="xpool", bufs=1) as xpool,
            tc.tile_pool(name="smalls", bufs=1) as smalls,
            tc.tile_pool(name="dram", bufs=1, space="DRAM") as dram,
        ):
            # ---- resident x^T (f32r) + fp8 variants ----
            xt_sb = xpool.tile([P, KC, TOK], F32R)
            x1q_sb = xpool.tile([P, KC, TOK], FP8)
            x2q_sb = xpool.tile([P, KC, TOK], FP8)

            e_sb = smalls.tile([2, P], BF16)
            nc.sync.dma_start(e_sb[:], e_sel.ap())
            ones_sb = smalls.tile([P, 1], F32)
            nc.vector.memset(ones_sb[:], 1.0)
            eps_sb = smalls.tile([P, 1], F32)
            nc.vector.memset(eps_sb[:], LN_EPS)
            # Per-head-pair block-diagonal KV operands (bf16, numerator only)
            # and Ksum columns (f32: the 1/(Q.Ksum) denominators cannot
            # afford f32r/bf16 operand rounding), filled after the AllReduce.
            kvbd = [smalls.tile([P, P], BF16, name=f"kvbd_{i}") for i in range(KC)]
            for kt in kvbd:
                nc.vector.memset(kt[:], 0.0)
            kd_sb = smalls.tile([P, H], F32)  # col h: Ksum_h at rows (h%2)*64
            nc.vector.memset(kd_sb[:], 0.0)
            acc = smalls.tile([P, 520], F32)
            nc.vector.memset(acc[:], 0.0)
            ar_sb = smalls.tile([P, 520], F32)
            if apply_bias:
                bq_sb = smalls.tile([P, KC], F32)   # per-partition layout for Q^T
                nc.sync.dma_start(bq_sb[:], bq_d.ap().rearrange("(ko p) -> p ko", p=P))
                bk_b = smalls.tile([P, D], F32)
                bv_b = smalls.tile([P, D], F32)
                bo_b = smalls.tile([P, D], F32)
                bcast_row(bk_d, bk_b[:])
                bcast_row(bv_d, bv_b[:])
                bcast_row(bo_d, bo_b[:])
            if apply_gamma:
                gamma_b = smalls.tile([P, D], F32)
                bcast_row(gamma_d, gamma_b[:])
            if apply_beta:
                beta_b = smalls.tile([P, D], F32)
                bcast_row(beta_d, beta_b[:])

            # PE warmup: matmuls on zeroed tiles hold the PE busy (p-state
            # ramp) while the first input DMAs are in flight.
            with (
                tc.tile_pool(name="warmsb", bufs=1) as warmsb,
                tc.tile_pool(name="warmps", bufs=2, space="PSUM") as warmps,
            ):
                warm_a = warmsb.tile([P, P], BF16)
                warm_b = warmsb.tile([P, 512], BF16)
                nc.gpsimd.memset(warm_a[:], 0.0)
                nc.gpsimd.memset(warm_b[:], 0.0)
                for w in range(WARMUP_MM):
                    wp = warmps.tile([P, 512], F32, tag="warm", name=f"warm_{w}")
                    nc.tensor.matmul(wp[:], warm_a[:], warm_b[:], start=True, stop=True)

            # Prefetch the first two Q-weight slices during phase 1.
            wqp_cm = tc.tile_pool(name="wqp", bufs=2)
            wqp = wqp_cm.__enter__()
            wq_tiles = {}

            def load_wq(hp):
                msl = slice(hp * P, (hp + 1) * P)
                wq_t = wqp.tile([P, KC, P], F32R, tag="wq", name=f"wq_{hp}")
                nc.sync.dma_start(wq_t[:], r8(wq_d)[:, :, msl])
                wq1_t = wqp.tile([P, KC, P], FP8, tag="wq1", name=f"wq1_{hp}")
                nc.sync.dma_start(wq1_t[:], r8(wq1_d)[:, :, msl])
                wq2_t = wqp.tile([P, KC, P], FP8, tag="wq2", name=f"wq2_{hp}")
                nc.sync.dma_start(wq2_t[:], r8(wq2_d)[:, :, msl])
                wq_tiles[hp] = (wq_t, wq1_t, wq2_t)

            # ================= Phase 1: K, V, partial KV + Ksum =================
            with (
                tc.tile_pool(name="wkv", bufs=1) as wkv,
                tc.tile_pool(name="kvps_pool", bufs=2, space="PSUM") as kvps_pool,
                tc.tile_pool(name="ph1ps", bufs=5, space="PSUM") as ph1ps,
                tc.tile_pool(name="ph1sb", bufs=3) as ph1sb,
            ):
                wk_sb = wkv.tile([P, KC, D], F32R)
                wv_sb = wkv.tile([P, KC, D], F32R)
                for k in range(KC):
                    nc.sync.dma_start(xt_sb[:, k, :], r8(xt)[:, k, :])
                    nc.sync.dma_start(wk_sb[:, k, :], r8(wk_d)[:, k, :])
                    nc.sync.dma_start(x1q_sb[:, k, :], r8(x1q_d)[:, k, :])
                    nc.sync.dma_start(wv_sb[:, k, :], r8(wv_d)[:, k, :])
                    nc.sync.dma_start(x2q_sb[:, k, :], r8(x2q_d)[:, k, :])
                load_wq(0)
                load_wq(1)

                for t in range(TC):
                    ts = slice(t * P, (t + 1) * P)
                    kb_chunks = []
                    kvs_tiles = {}
                    for dh in range(2):
                        dsl = slice(dh * 512, (dh + 1) * 512)
                        kps = ph1ps.tile([P, 512], F32, tag="proj", name=f"kps_{t}_{dh}")
                        for k in range(KC):
                            nc.tensor.matmul(kps[:], xt_sb[:, k, ts], wk_sb[:, k, dsl],
                                             start=(k == 0), stop=(k == KC - 1))
                        if apply_bias:
                            kraw = ph1sb.tile([P, 512], F32, tag="kraw", name=f"kraw_{t}_{dh}")
                            nc.vector.tensor_tensor(kraw[:], kps[:], bk_b[:, dsl], OP.add)
                            ksrc = kraw
                        else:
                            ksrc = kps
                        # ELU = max(exp(min(x,0)) - 1, x); min(x,0) = -relu(-x)
                        kmin = ph1sb.tile([P, 512], F32, tag="kmin", name=f"kmin_{t}_{dh}")
                        nc.scalar.activation(kmin[:], ksrc[:], AF.Relu, scale=-1.0)
                        kexp = ph1sb.tile([P, 512], F32, tag="kexp", name=f"kexp_{t}_{dh}")
                        nc.scalar.activation(kexp[:], kmin[:], AF.Exp, scale=-1.0)
                        kf = ph1sb.tile([P, 512], F32, tag="kf", name=f"kf_{t}_{dh}")
                        nc.vector.scalar_tensor_tensor(kf[:], kexp[:], -1.0, ksrc[:],
                                                       OP.add, OP.max)
                        kb = ph1sb.tile([P, 512], BF16, tag="kb", name=f"kb_{t}_{dh}")
                        nc.gpsimd.tensor_copy(kb[:], kf[:])  # SBUF->SBUF cast
                        kb_chunks.append(kb)
                        # Ksum column blocks (f32 matmul against ones) go into
                        # cols [256, 260) of the shared kvs_t psum tile.
                        kvs_t = kvps_pool.tile([P, 260], F32, tag="kvs_t",
                                               name=f"kvs_t_{t}_{dh}")
                        kvs_tiles[dh] = kvs_t
                        for j in range(4):
                            nc.tensor.matmul(
                                kvs_t[:, 256 + j:257 + j], kf[:, j * P:(j + 1) * P],
                                ones_sb[:], start=True, stop=True, skip_group_check=True)
                    for dh in range(2):
                        dsl = slice(dh * 512, (dh + 1) * 512)
                        vps = ph1ps.tile([P, 512], F32, tag="proj", name=f"vps_{t}_{dh}")
                        for k in range(KC):
                            nc.tensor.matmul(vps[:], xt_sb[:, k, ts], wv_sb[:, k, dsl],
                                             start=(k == 0), stop=(k == KC - 1))
                        vb = ph1sb.tile([P, 512], BF16, tag="vb", name=f"vb_{t}_{dh}")
                        if apply_bias:
                            nc.vector.tensor_tensor(vb[:], vps[:], bv_b[:, dsl], OP.add)
                        else:
                            nc.vector.tensor_copy(vb[:], vps[:])
                        kb = kb_chunks[dh]
                        kvs_t = kvs_tiles[dh]
                        for hh in range(8):
                            h = dh * 8 + hh
                            pr = (h % 2) * 64
                            fc = (h // 2) * 64 - dh * 256
                            nc.tensor.matmul(
                                kvs_t[pr:pr + 64, fc:fc + 64],
                                kb[:, hh * 64:(hh + 1) * 64],
                                vb[:, hh * 64:(hh + 1) * 64],
                                start=True, stop=True,
                                tile_position=(0, pr), skip_group_check=True)
                        nc.vector.tensor_tensor(
                            acc[:, dh * 260:(dh + 1) * 260],
                            acc[:, dh * 260:(dh + 1) * 260], kvs_t[:], OP.add)

            # ========== Phases 2-4: AllReduce; Q^T; attention ==========
            with tc.tile_pool(name="late", bufs=1) as late:
                at_sb = late.tile([P, KC, TOK], BF16)

                # -- AllReduce of the packed KV/Ksum accumulator --
                cc_in = dram.tile([P, 520], F32)
                cc_out = dram.tile([P, 520], F32)
                nc.sync.dma_start(cc_in[:], acc[:])
                nc.gpsimd.collective_compute(
                    "AllReduce", OP.add,
                    replica_groups=[[0, 1], [2, 3], [4, 5], [6, 7]],
                    ins=[cc_in[:].opt()], outs=[cc_out[:].opt()])
                nc.sync.dma_start(ar_sb[:], cc_out[:])

                with (
                    tc.tile_pool(name="qtp", bufs=4) as qtp,
                    tc.tile_pool(name="ph3ps", bufs=3, space="PSUM") as ph3ps,
                    tc.tile_pool(name="ph3psc", bufs=1, space="PSUM") as ph3psc,
                    tc.tile_pool(name="ph3sb", bufs=2) as ph3sb,
                    tc.tile_pool(name="ph4ps_d", bufs=1, space="PSUM") as ph4ps_d,
                    tc.tile_pool(name="ph4ps_z", bufs=1, space="PSUM") as ph4ps_z,
                    tc.tile_pool(name="ph4ps_a", bufs=2, space="PSUM") as ph4ps_a,
                    tc.tile_pool(name="ph4sb", bufs=2) as ph4sb,
                ):
                    qt_tiles = {}

                    def q_proj(hp):
                        wq_t, wq1_t, wq2_t = wq_tiles.pop(hp)
                        qt = qtp.tile([P, TOK], F32, tag="qt", name=f"qt_{hp}")
                        qt_tiles[hp] = qt
                        for tq in range(TQ):
                            tsl = slice(tq * 512, (tq + 1) * 512)
                            qps = ph3ps.tile([P, 512], F32, tag="qps", name=f"qps_{hp}_{tq}")
                            for k in range(KC):
                                nc.tensor.matmul(qps[:], wq_t[:, k, :], xt_sb[:, k, tsl],
                                                 start=(k == 0), stop=(k == KC - 1))
                            qsum = ph3sb.tile([P, 512], F32, tag="qsum", name=f"qsum_{hp}_{tq}")
                            if Q_CORR:
                                # fp8 DoubleRow correction: 2^22*(x_lo@Wq + x@Wq_lo)
                                cps = ph3psc.tile([P, 512], F32, tag="cps", name=f"cps_{hp}_{tq}")
                                for k2 in range(KC // 2):
                                    nc.tensor.matmul(cps[:], wq1_t[:, 2 * k2:2 * k2 + 2, :],
                                                     x2q_sb[:, 2 * k2:2 * k2 + 2, tsl],
                                                     start=(k2 == 0), stop=False, perf_mode=DR)
                                for k2 in range(KC // 2):
                                    nc.tensor.matmul(cps[:], wq2_t[:, 2 * k2:2 * k2 + 2, :],
                                                     x1q_sb[:, 2 * k2:2 * k2 + 2, tsl],
                                                     start=False, stop=(k2 == KC // 2 - 1),
                                                     perf_mode=DR)
                                cc = ph3sb.tile([P, 512], F32, tag="cc", name=f"cc_{hp}_{tq}")
                                nc.scalar.activation(cc[:], cps[:], AF.Copy)
                                nc.vector.scalar_tensor_tensor(qsum[:], cc[:], QC_DESCALE,
                                                               qps[:], OP.mult, OP.add)
                            else:
                                nc.vector.tensor_copy(qsum[:], qps[:])
                            if apply_bias:
                                nc.vector.tensor_scalar(qsum[:], qsum[:], bq_sb[:, hp:hp + 1],
                                                        None, OP.add)
                            qmin = ph3sb.tile([P, 512], F32, tag="qmin", name=f"qmin_{hp}_{tq}")
                            nc.scalar.activation(qmin[:], qsum[:], AF.Relu, scale=-1.0)
                            qexp = ph3sb.tile([P, 512], F32, tag="qexp", name=f"qexp_{hp}_{tq}")
                            nc.scalar.activation(qexp[:], qmin[:], AF.Exp, scale=-1.0)
                            nc.vector.scalar_tensor_tensor(qt[:, tsl], qexp[:], -1.0,
                                                           qsum[:], OP.add, OP.max)

                    def attention(hp):
                        qt = qt_tiles.pop(hp)
                        for tq in range(TQ):
                            tsl = slice(tq * 512, (tq + 1) * 512)
                            dps = ph4ps_d.tile([2, 512], F32, tag="dps", name=f"dps_{hp}_{tq}")
                            nc.tensor.matmul(dps[:], kd_sb[:, 2 * hp:2 * hp + 2],
                                             qt[:, tsl], start=True, stop=True)
                            zr = ph4sb.tile([2, 512], BF16, tag="zr", name=f"zr_{hp}_{tq}")
                            with nc.allow_low_precision(reason="Z is a per-token scale"):
                                nc.vector.reciprocal(zr[:], dps[:])
                            zps = ph4ps_z.tile([P, 512], F32, tag="zps", name=f"zps_{hp}_{tq}")
                            nc.tensor.matmul(zps[:], e_sb[:], zr[:], start=True, stop=True)
                            zf = ph4sb.tile([P, 512], BF16, tag="zf", name=f"zf_{hp}_{tq}")
                            nc.scalar.activation(zf[:], zps[:], AF.Copy)
                            qb = ph4sb.tile([P, 512], BF16, tag="qb", name=f"qb_{hp}_{tq}")
                            nc.gpsimd.tensor_copy(qb[:], qt[:, tsl])
                            aps = ph4ps_a.tile([P, 512], F32, tag="aps", name=f"aps_{hp}_{tq}")
                            nc.tensor.matmul(aps[:], kvbd[hp][:], qb[:],
                                             start=True, stop=True)
                            nc.vector.tensor_tensor(at_sb[:, hp, tsl], aps[:], zf[:], OP.mult)

                    # Emit every q_proj before the first attention so the PE
                    # queue never stalls behind AllReduce-dependent work.
                    for hp in range(KC):
                        if 2 <= hp + 1 < KC:
                            load_wq(hp + 1)
                        q_proj(hp)

                    # Unpack AllReduce result into f32 KV / Ksum operands
                    # (tiles pre-zeroed at allocation; waits on ar_sb).
                    for hp in range(KC):
                        off = (hp // 4) * 260 + (hp % 4) * 64
                        nc.vector.tensor_copy(kvbd[hp][0:64, 0:64],
                                              ar_sb[0:64, off:off + 64])
                        nc.vector.tensor_copy(kvbd[hp][64:P, 64:P],
                                              ar_sb[64:P, off:off + 64])
                    for h in range(H):
                        pr = (h % 2) * 64
                        c = h // 2
                        sc = 256 + c if c < 4 else 516 + (c - 4)
                        nc.vector.tensor_copy(
                            kd_sb[pr:pr + 64, h:h + 1], ar_sb[pr:pr + 64, sc:sc + 1])

                    for hp in range(KC):
                        attention(hp)

                # ===== Phase 5: output projection + residual + LayerNorm =====
                with (
                    tc.tile_pool(name="wop", bufs=1) as wop,
                    tc.tile_pool(name="ph5ps", bufs=3, space="PSUM") as ph5ps,
                    tc.tile_pool(name="ph5sb", bufs=3) as ph5sb,
                ):
                    wo_sb = wop.tile([P, KC, D], BF16)
                    for k in range(KC):
                        nc.sync.dma_start(wo_sb[:, k, :], r8(wo_d)[:, k, :])
                    for t in range(TC):
                        ts = slice(t * P, (t + 1) * P)
                        y = ph5sb.tile([P, D], F32, tag="y", name=f"y_{t}")
                        xr = ph5sb.tile([P, D], F32, tag="xr", name=f"xr_{t}")
                        nc.sync.dma_start(xr[:], xres.ap()[ts, :])
                        ops = ph5ps.tile([P, D], F32, tag="ops", name=f"ops_{t}")
                        for dh in range(2):
                            dsl = slice(dh * 512, (dh + 1) * 512)
                            for c in range(KC):
                                nc.tensor.matmul(ops[:, dsl], at_sb[:, c, ts], wo_sb[:, c, dsl],
                                                 start=(c == 0), stop=(c == KC - 1))
                        nc.vector.tensor_tensor(y[:], ops[:], xr[:], OP.add)
                        if apply_bias:
                            nc.vector.tensor_tensor(y[:], y[:], bo_b[:], OP.add)
                        stats = ph5sb.tile([P, 2, 6], F32, tag="stats", name=f"stats_{t}")
                        nc.vector.bn_stats(out=stats[:, 0, :], in_=y[:, :512])
                        nc.vector.bn_stats(out=stats[:, 1, :], in_=y[:, 512:])
                        mv = ph5sb.tile([P, 2], F32, tag="mv", name=f"mv_{t}")
                        nc.vector.bn_aggr(out=mv[:], in_=stats[:])
                        nc.scalar.activation(out=mv[:, 1:2], in_=mv[:, 1:2], func=AF.Sqrt,
                                             bias=eps_sb[:], scale=1.0)
                        nc.vector.reciprocal(mv[:, 1:2], mv[:, 1:2])
                        yo = ph5sb.tile([P, D], F32, tag="yo", name=f"yo_{t}")
                        nc.gpsimd.tensor_scalar(yo[:], y[:], mv[:, 0:1], mv[:, 1:2],
                                                OP.subtract, OP.mult)
                        if apply_gamma:
                            nc.vector.tensor_tensor(yo[:], yo[:], gamma_b[:], OP.mult)
                        if apply_beta:
                            nc.vector.tensor_tensor(yo[:], yo[:], beta_b[:], OP.add)
                        nc.sync.dma_start(out_d.ap()[ts, :], yo[:])

            wqp_cm.__exit__(None, None, None)

    nc.compile()
    return nc


def kernel(x, Wq, bq, Wk, bk, Wv, bv, Wo, bo, gamma, beta):
    global LAST_RESULT
    x = np.asarray(x, dtype=np.float32)
    f32 = np.float32
    bf16 = ml_dtypes.bfloat16

    apply_bias = any(np.any(np.asarray(b)) for b in (bq, bk, bv, bo))
    apply_gamma = not np.all(np.asarray(gamma) == 1.0)
    apply_beta = bool(np.any(np.asarray(beta)))

    nc = _build(apply_bias, apply_gamma, apply_beta)

    e4 = ml_dtypes.float8_e4m3

    def rn11(a):
        ai = np.ascontiguousarray(a, dtype=f32).view(np.uint32)
        keep = np.uint32(0xFFFFFFFF) << np.uint32(12)
        half = np.uint32(1) << np.uint32(11)
        return ((ai + half) & keep).view(f32)

    wq = np.asarray(Wq, f32)
    wk = np.asarray(Wk, f32)
    wv = np.asarray(Wv, f32)
    wo = np.asarray(Wo, f32).astype(bf16)
    wq1 = (wq * 2.0 ** 10).astype(e4)
    wq2 = ((wq - rn11(wq)) * 2.0 ** 16).astype(e4)
    e_sel = np.zeros((2, P), dtype=bf16)
    e_sel[0, :64] = 1
    e_sel[1, 64:] = 1

    in_maps = []
    for c in range(NCORES):
        b, half = c // 2, c % 2
        xs = x[b, half * TOK:(half + 1) * TOK]          # [2048, 1024]
        xst = np.ascontiguousarray(xs.T)
        m = {
            "xt": xst,
            "x1q": (xst * 2.0 ** 5).astype(e4),
            "x2q": ((xst - rn11(xst)) * 2.0 ** 11).astype(e4),
            "wq": wq, "wk": wk, "wq1": wq1, "wq2": wq2, "wv": wv, "wo": wo,
            "xres": np.ascontiguousarray(xs),
            "e_sel": e_sel,
        }
        if apply_bias:
            m.update(bq=np.asarray(bq, f32), bk=np.asarray(bk, f32),
                     bv=np.asarray(bv, f32), bo=np.asarray(bo, f32))
        if apply_gamma:
            m["gamma"] = np.asarray(gamma, f32)
        if apply_beta:
            m["beta"] = np.asarray(beta, f32)
        in_maps.append(m)

    import os
    try:
        LAST_RESULT = run_bass_kernel_spmd(nc, in_maps, core_ids=list(range(NCORES)))
    except ModuleNotFoundError:
        # no antenv.axon_hooks in this container -> NTFF tracing unavailable
        os.environ["BASS_NEVER_TRACE"] = "1"
        LAST_RESULT = run_bass_kernel_spmd(nc, in_maps, core_ids=list(range(NCORES)))
    out = np.empty((B, N, D), dtype=np.float32)
    for c in range(NCORES):
        b, half = c // 2, c % 2
        out[b, half * TOK:(half + 1) * TOK] = LAST_RESULT.results[c]["out"]
    return out


# revision 52
# speedup vs baseline: 1.1414x; 1.1414x over previous
"""Distributed Trainium2 kernel for the linear-attention transformer block.

Math (per batch element b):
  Q = elu(x @ Wq), K = elu(x @ Wk), V = x @ Wv   (per-head d=64)
  KV_h = K_h^T V_h  [64,64];  Ksum_h = sum_n K_h[n]  [64]
  attn_h = (Q_h @ KV_h) / (Q_h . Ksum_h)
  out = LayerNorm(x + attn @ Wo) * gamma + beta

Sharding: 16384 tokens over 8 cores (2048 each; core c owns batch c//2,
half c%2). Each core computes Q/K/V only for its tokens, partial KV/Ksum,
then a 266KB AllReduce over core pairs {2b, 2b+1} completes the KV stats;
attention + output projection + LayerNorm finish locally.

Precision: projections run in f32r (full-rate on the PE for moving dims
>=256, ~17-bit effective mantissa), which keeps the 1/(Q.Ksum)
denominators accurate without multi-term bf16 splits. KV/attention
numerator uses bf16 where it only feeds the (residual-dominated) output.
ELU is computed as max(exp(min(x,0)) - 1, x) - one op per engine stage
(Pool min, ACT exp, DVE fused scalar_tensor_tensor).
"""

import sys

sys.path.insert(0, "/opt/trn_rl_repo")

import numpy as np
import ml_dtypes

import concourse.bass as bass
import concourse.mybir as mybir
import concourse.tile as tile
from concourse import bacc
from concourse.bass_utils import run_bass_kernel_spmd

AF = mybir.ActivationFunctionType
OP = mybir.AluOpType
F32 = mybir.dt.float32
F32R = mybir.dt.float32r
BF16 = mybir.dt.bfloat16
FP8 = mybir.dt.float8e4   # IEEE e4m3: max finite 240 (NOT the fn variant)
DR = mybir.MatmulPerfMode.DoubleRow
QC_DESCALE = 2.0 ** -21   # x2q(2^11)@wq1(2^10) == x1q(2^5)@wq2(2^16)
V_DESCALE = 2.0 ** -15    # x1q(2^5) @ wv8(2^10)

B, N, D = 4, 4096, 1024
H, HD = 16, 64
TOK = 2048            # tokens per core
NCORES = 8
LN_EPS = 1e-3
P = 128
KC = D // P           # 8 contraction chunks
TC = TOK // P         # 16 token chunks of 128
TQ = TOK // 512       # 4 token chunks of 512

WARMUP_MM = 64
Q_CORR = True     # fp8 DoubleRow correction terms on the Q projection

LAST_RESULT = None    # BassKernelResults of the most recent run (for test.py)


def _build(apply_bias, apply_gamma, apply_beta):
    nc = bacc.Bacc("TRN2", target_bir_lowering=False, debug=False, num_devices=NCORES)

    xt = nc.dram_tensor("xt", [D, TOK], F32R, kind="ExternalInput")
    wq_d = nc.dram_tensor("wq", [D, D], F32R, kind="ExternalInput")
    wk_d = nc.dram_tensor("wk", [D, D], F32R, kind="ExternalInput")
    # fp8 operands: x1q = x*2^5, x2q = (x - rn11(x))*2^11, wq1 = Wq*2^10,
    # wq2 = (Wq - rn11(Wq))*2^16 (all e4m3, host-quantized).
    x1q_d = nc.dram_tensor("x1q", [D, TOK], FP8, kind="ExternalInput")
    x2q_d = nc.dram_tensor("x2q", [D, TOK], FP8, kind="ExternalInput")
    wq1_d = nc.dram_tensor("wq1", [D, D], FP8, kind="ExternalInput")
    wq2_d = nc.dram_tensor("wq2", [D, D], FP8, kind="ExternalInput")
    wv_d = nc.dram_tensor("wv", [D, D], F32R, kind="ExternalInput")
    wo_d = nc.dram_tensor("wo", [D, D], BF16, kind="ExternalInput")
    xres = nc.dram_tensor("xres", [TOK, D], F32, kind="ExternalInput")
    e_sel = nc.dram_tensor("e_sel", [2, P], BF16, kind="ExternalInput")
    if apply_bias:
        bq_d = nc.dram_tensor("bq", [D], F32, kind="ExternalInput")
        bk_d = nc.dram_tensor("bk", [D], F32, kind="ExternalInput")
        bv_d = nc.dram_tensor("bv", [D], F32, kind="ExternalInput")
        bo_d = nc.dram_tensor("bo", [D], F32, kind="ExternalInput")
    if apply_gamma:
        gamma_d = nc.dram_tensor("gamma", [D], F32, kind="ExternalInput")
    if apply_beta:
        beta_d = nc.dram_tensor("beta", [D], F32, kind="ExternalInput")
    out_d = nc.dram_tensor("out", [TOK, D], F32, kind="ExternalOutput")

    r8 = lambda t: t.ap().rearrange("(ko p) n -> p ko n", p=P)

    def bcast_row(dram_vec, sb_tile):
        # DMA-broadcast a [D] vector to [P, D] (stride-0 partition dim).
        src = bass.AP(
            tensor=dram_vec.ap().tensor,
            offset=dram_vec.ap().offset,
            ap=[[0, P]] + list(dram_vec.ap().ap),
        )
        nc.sync.dma_start(out=sb_tile, in_=src)

    with tile.TileContext(nc) as tc:
        with (
            tc.tile_pool(name="xpool", bufs=1) as xpool,
            tc.tile_pool(name="smalls", bufs=1) as smalls,
            tc.tile_pool(name="dram", bufs=1, space="DRAM") as dram,
        ):
            # ---- resident x^T (f32r) + fp8 variants ----
            xt_sb = xpool.tile([P, KC, TOK], F32R)
            x1q_sb = xpool.tile([P, KC, TOK], FP8)
            x2q_sb = xpool.tile([P, KC, TOK], FP8)

            e_sb = smalls.tile([2, P], BF16)
            nc.sync.dma_start(e_sb[:], e_sel.ap())
            ones_sb = smalls.tile([P, 1], F32)
            nc.vector.memset(ones_sb[:], 1.0)
            eps_sb = smalls.tile([P, 1], F32)
            nc.vector.memset(eps_sb[:], LN_EPS)
            # Per-head-pair block-diagonal KV operands (bf16, numerator only)
            # and Ksum columns (f32: the 1/(Q.Ksum) denominators cannot
            # afford f32r/bf16 operand rounding), filled after the AllReduce.
            kvbd = [smalls.tile([P, P], BF16, name=f"kvbd_{i}") for i in range(KC)]
            for kt in kvbd:
                nc.vector.memset(kt[:], 0.0)
            kd_sb = smalls.tile([P, H], F32)  # col h: Ksum_h at rows (h%2)*64
            nc.vector.memset(kd_sb[:], 0.0)
            acc = smalls.tile([P, 520], F32)
            nc.vector.memset(acc[:], 0.0)
            ar_sb = smalls.tile([P, 520], F32)
            if apply_bias:
                bq_sb = smalls.tile([P, KC], F32)   # per-partition layout for Q^T
                nc.sync.dma_start(bq_sb[:], bq_d.ap().rearrange("(ko p) -> p ko", p=P))
                bk_b = smalls.tile([P, D], F32)
                bv_b = smalls.tile([P, D], F32)
                bo_b = smalls.tile([P, D], F32)
                bcast_row(bk_d, bk_b[:])
                bcast_row(bv_d, bv_b[:])
                bcast_row(bo_d, bo_b[:])
            if apply_gamma:
                gamma_b = smalls.tile([P, D], F32)
                bcast_row(gamma_d, gamma_b[:])
            if apply_beta:
                beta_b = smalls.tile([P, D], F32)
                bcast_row(beta_d, beta_b[:])

            # PE warmup: matmuls on zeroed tiles hold the PE busy (p-state
            # ramp) while the first input DMAs are in flight.
            with (
                tc.tile_pool(name="warmsb", bufs=1) as warmsb,
                tc.tile_pool(name="warmps", bufs=2, space="PSUM") as warmps,
            ):
                warm_a = warmsb.tile([P, P], BF16)
                warm_b = warmsb.tile([P, 512], BF16)
                nc.gpsimd.memset(warm_a[:], 0.0)
                nc.gpsimd.memset(warm_b[:], 0.0)
                for w in range(WARMUP_MM):
                    wp = warmps.tile([P, 512], F32, tag="warm", name=f"warm_{w}")
                    nc.tensor.matmul(wp[:], warm_a[:], warm_b[:], start=True, stop=True)

            # Prefetch the first two Q-weight slices during phase 1.
            wqp_cm = tc.tile_pool(name="wqp", bufs=2)
            wqp = wqp_cm.__enter__()
            wq_tiles = {}

            def load_wq(hp):
                msl = slice(hp * P, (hp + 1) * P)
                wq_t = wqp.tile([P, KC, P], F32R, tag="wq", name=f"wq_{hp}")
                nc.sync.dma_start(wq_t[:], r8(wq_d)[:, :, msl])
                wq1_t = wqp.tile([P, KC, P], FP8, tag="wq1", name=f"wq1_{hp}")
                nc.sync.dma_start(wq1_t[:], r8(wq1_d)[:, :, msl])
                wq2_t = wqp.tile([P, KC, P], FP8, tag="wq2", name=f"wq2_{hp}")
                nc.sync.dma_start(wq2_t[:], r8(wq2_d)[:, :, msl])
                wq_tiles[hp] = (wq_t, wq1_t, wq2_t)

            # ================= Phase 1: K, V, partial KV + Ksum =================
            with (
                tc.tile_pool(name="wkv", bufs=1) as wkv,
                tc.tile_pool(name="kvps_pool", bufs=2, space="PSUM") as kvps_pool,
                tc.tile_pool(name="ph1ps", bufs=5, space="PSUM") as ph1ps,
                tc.tile_pool(name="ph1sb", bufs=3) as ph1sb,
            ):
                wk_sb = wkv.tile([P, KC, D], F32R)
                wv_sb = wkv.tile([P, KC, D], F32R)
                # The first K psum group needs ALL xt+wk chunks; stream those
                # first, then wv (needed by t=0's V loop), then the fp8 x
                # copies (phase 3 only).
                for k in range(KC):
                    nc.sync.dma_start(xt_sb[:, k, :], r8(xt)[:, k, :])
                    nc.sync.dma_start(wk_sb[:, k, :], r8(wk_d)[:, k, :])
                for k in range(KC):
                    nc.sync.dma_start(wv_sb[:, k, :], r8(wv_d)[:, k, :])
                load_wq(0)
                load_wq(1)
                for k in range(KC):
                    nc.sync.dma_start(x1q_sb[:, k, :], r8(x1q_d)[:, k, :])
                    nc.sync.dma_start(x2q_sb[:, k, :], r8(x2q_d)[:, k, :])

                for t in range(TC):
                    ts = slice(t * P, (t + 1) * P)
                    kb_chunks = []
                    kvs_tiles = {}
                    for dh in range(2):
                        dsl = slice(dh * 512, (dh + 1) * 512)
                        kps = ph1ps.tile([P, 512], F32, tag="proj", name=f"kps_{t}_{dh}")
                        for k in range(KC):
                            nc.tensor.matmul(kps[:], xt_sb[:, k, ts], wk_sb[:, k, dsl],
                                             start=(k == 0), stop=(k == KC - 1))
                        if apply_bias:
                            kraw = ph1sb.tile([P, 512], F32, tag="kraw", name=f"kraw_{t}_{dh}")
                            nc.vector.tensor_tensor(kraw[:], kps[:], bk_b[:, dsl], OP.add)
                            ksrc = kraw
                        else:
                            ksrc = kps
                        # ELU = max(exp(min(x,0)) - 1, x); min(x,0) = -relu(-x)
                        kmin = ph1sb.tile([P, 512], F32, tag="kmin", name=f"kmin_{t}_{dh}")
                        nc.scalar.activation(kmin[:], ksrc[:], AF.Relu, scale=-1.0)
                        kexp = ph1sb.tile([P, 512], F32, tag="kexp", name=f"kexp_{t}_{dh}")
                        nc.scalar.activation(kexp[:], kmin[:], AF.Exp, scale=-1.0)
                        kf = ph1sb.tile([P, 512], F32, tag="kf", name=f"kf_{t}_{dh}")
                        nc.vector.scalar_tensor_tensor(kf[:], kexp[:], -1.0, ksrc[:],
                                                       OP.add, OP.max)
                        kb = ph1sb.tile([P, 512], BF16, tag="kb", name=f"kb_{t}_{dh}")
                        nc.gpsimd.tensor_copy(kb[:], kf[:])  # SBUF->SBUF cast
                        kb_chunks.append(kb)
                        # Ksum column blocks (f32 matmul against ones) go into
                        # cols [256, 260) of the shared kvs_t psum tile.
                        kvs_t = kvps_pool.tile([P, 260], F32, tag="kvs_t",
                                               name=f"kvs_t_{t}_{dh}")
                        kvs_tiles[dh] = kvs_t
                        for j in range(4):
                            nc.tensor.matmul(
                                kvs_t[:, 256 + j:257 + j], kf[:, j * P:(j + 1) * P],
                                ones_sb[:], start=True, stop=True, skip_group_check=True)
                    for dh in range(2):
                        dsl = slice(dh * 512, (dh + 1) * 512)
                        vps = ph1ps.tile([P, 512], F32, tag="proj", name=f"vps_{t}_{dh}")
                        for k in range(KC):
                            nc.tensor.matmul(vps[:], xt_sb[:, k, ts], wv_sb[:, k, dsl],
                                             start=(k == 0), stop=(k == KC - 1))
                        vb = ph1sb.tile([P, 512], BF16, tag="vb", name=f"vb_{t}_{dh}")
                        if apply_bias:
                            nc.vector.tensor_tensor(vb[:], vps[:], bv_b[:, dsl], OP.add)
                        else:
                            nc.vector.tensor_copy(vb[:], vps[:])
                        kb = kb_chunks[dh]
                        kvs_t = kvs_tiles[dh]
                        for hh in range(8):
                            h = dh * 8 + hh
                            pr = (h % 2) * 64
                            fc = (h // 2) * 64 - dh * 256
                            nc.tensor.matmul(
                                kvs_t[pr:pr + 64, fc:fc + 64],
                                kb[:, hh * 64:(hh + 1) * 64],
                                vb[:, hh * 64:(hh + 1) * 64],
                                start=True, stop=True,
                                tile_position=(0, pr), skip_group_check=True)
                        nc.vector.tensor_tensor(
                            acc[:, dh * 260:(dh + 1) * 260],
                            acc[:, dh * 260:(dh + 1) * 260], kvs_t[:], OP.add)

            # ========== Phases 2-4: AllReduce; Q^T; attention ==========
            with tc.tile_pool(name="late", bufs=1) as late:
                at_sb = late.tile([P, KC, TOK], BF16)

                # -- AllReduce of the packed KV/Ksum accumulator --
                cc_in = dram.tile([P, 520], F32)
                cc_out = dram.tile([P, 520], F32)
                nc.sync.dma_start(cc_in[:], acc[:])
                nc.gpsimd.collective_compute(
                    "AllReduce", OP.add,
                    replica_groups=[[0, 1], [2, 3], [4, 5], [6, 7]],
                    ins=[cc_in[:].opt()], outs=[cc_out[:].opt()])
                nc.sync.dma_start(ar_sb[:], cc_out[:])

                with tc.tile_pool(name="qtp", bufs=4) as qtp:
                    qt_tiles = {}

                    def q_proj(hp, ph3ps, ph3psc, ph3sb):
                        wq_t, wq1_t, wq2_t = wq_tiles.pop(hp)
                        qt = qtp.tile([P, TOK], F32, tag="qt", name=f"qt_{hp}")
                        qt_tiles[hp] = qt
                        for tq in range(TQ):
                            tsl = slice(tq * 512, (tq + 1) * 512)
                            qps = ph3ps.tile([P, 512], F32, tag="qps", name=f"qps_{hp}_{tq}")
                            for k in range(KC):
                                nc.tensor.matmul(qps[:], wq_t[:, k, :], xt_sb[:, k, tsl],
                                                 start=(k == 0), stop=(k == KC - 1))
                            qsum = ph3sb.tile([P, 512], F32, tag="qsum", name=f"qsum_{hp}_{tq}")
                            if Q_CORR:
                                # fp8 DoubleRow correction: 2^22*(x_lo@Wq + x@Wq_lo)
                                cps = ph3psc.tile([P, 512], F32, tag="cps", name=f"cps_{hp}_{tq}")
                                for k2 in range(KC // 2):
                                    nc.tensor.matmul(cps[:], wq1_t[:, 2 * k2:2 * k2 + 2, :],
                                                     x2q_sb[:, 2 * k2:2 * k2 + 2, tsl],
                                                     start=(k2 == 0), stop=False, perf_mode=DR)
                                for k2 in range(KC // 2):
                                    nc.tensor.matmul(cps[:], wq2_t[:, 2 * k2:2 * k2 + 2, :],
                                                     x1q_sb[:, 2 * k2:2 * k2 + 2, tsl],
                                                     start=False, stop=(k2 == KC // 2 - 1),
                                                     perf_mode=DR)
                                cc = ph3sb.tile([P, 512], F32, tag="cc", name=f"cc_{hp}_{tq}")
                                nc.scalar.activation(cc[:], cps[:], AF.Copy)
                                nc.vector.scalar_tensor_tensor(qsum[:], cc[:], QC_DESCALE,
                                                               qps[:], OP.mult, OP.add)
                            else:
                                nc.vector.tensor_copy(qsum[:], qps[:])
                            if apply_bias:
                                nc.vector.tensor_scalar(qsum[:], qsum[:], bq_sb[:, hp:hp + 1],
                                                        None, OP.add)
                            qmin = ph3sb.tile([P, 512], F32, tag="qmin", name=f"qmin_{hp}_{tq}")
                            nc.scalar.activation(qmin[:], qsum[:], AF.Relu, scale=-1.0)
                            qexp = ph3sb.tile([P, 512], F32, tag="qexp", name=f"qexp_{hp}_{tq}")
                            nc.scalar.activation(qexp[:], qmin[:], AF.Exp, scale=-1.0)
                            nc.vector.scalar_tensor_tensor(qt[:, tsl], qexp[:], -1.0,
                                                           qsum[:], OP.add, OP.max)

                    def attention(hp, ph4ps_d, ph4ps_z, ph4ps_a, ph4sb):
                        qt = qt_tiles.pop(hp)
                        qbs = []
                        for tq in range(TQ):
                            # qb casts are off the dps->at critical chain;
                            # emit first so Pool runs ahead.
                            qb = ph4sb.tile([P, 512], BF16, tag="qb",
                                            name=f"qb_{hp}_{tq}")
                            nc.gpsimd.tensor_copy(qb[:], qt[:, tq * 512:(tq + 1) * 512])
                            qbs.append(qb)
                        for tq in range(TQ):
                            tsl = slice(tq * 512, (tq + 1) * 512)
                            dps = ph4ps_d.tile([2, 512], F32, tag="dps", name=f"dps_{hp}_{tq}")
                            nc.tensor.matmul(dps[:], kd_sb[:, 2 * hp:2 * hp + 2],
                                             qt[:, tsl], start=True, stop=True)
                            zr = ph4sb.tile([2, 512], BF16, tag="zr", name=f"zr_{hp}_{tq}")
                            with nc.allow_low_precision(reason="Z is a per-token scale"):
                                nc.vector.reciprocal(zr[:], dps[:])
                            zps = ph4ps_z.tile([P, 512], F32, tag="zps", name=f"zps_{hp}_{tq}")
                            nc.tensor.matmul(zps[:], e_sb[:], zr[:], start=True, stop=True)
                            zf = ph4sb.tile([P, 512], BF16, tag="zf", name=f"zf_{hp}_{tq}")
                            nc.scalar.activation(zf[:], zps[:], AF.Copy)
                            aps = ph4ps_a.tile([P, 512], F32, tag="aps", name=f"aps_{hp}_{tq}")
                            nc.tensor.matmul(aps[:], kvbd[hp][:], qbs[tq][:],
                                             start=True, stop=True)
                            nc.vector.tensor_tensor(at_sb[:, hp, tsl], aps[:], zf[:], OP.mult)

                    # Two rounds of (4 q_projs -> 4 attentions). Within each
                    # round the phase-3 PSUM pools close before attention so
                    # the dps/zps/aps chain gets deep buffering; the round
                    # split keeps the qt pool (4 bufs) cycle-free.
                    for rnd in range(2):
                        hps = range(4 * rnd, 4 * rnd + 4)
                        with (
                            tc.tile_pool(name=f"ph3ps_{rnd}", bufs=3, space="PSUM") as ph3ps,
                            tc.tile_pool(name=f"ph3psc_{rnd}", bufs=1, space="PSUM") as ph3psc,
                            tc.tile_pool(name=f"ph3sb_{rnd}", bufs=2) as ph3sb,
                        ):
                            for hp in hps:
                                if 2 <= hp + 1 < KC:
                                    load_wq(hp + 1)
                                q_proj(hp, ph3ps, ph3psc, ph3sb)
                        with (
                            tc.tile_pool(name=f"ph4ps_d_{rnd}", bufs=2, space="PSUM") as ph4ps_d,
                            tc.tile_pool(name=f"ph4ps_z_{rnd}", bufs=2, space="PSUM") as ph4ps_z,
                            tc.tile_pool(name=f"ph4ps_a_{rnd}", bufs=3, space="PSUM") as ph4ps_a,
                            tc.tile_pool(name=f"ph4sb_{rnd}", bufs=4) as ph4sb,
                        ):
                            if rnd == 0:
                                # Unpack AllReduce result into KV/Ksum
                                # operands, spread across engines.
                                for hp in range(KC):
                                    off = (hp // 4) * 260 + (hp % 4) * 64
                                    nc.scalar.copy(kvbd[hp][0:64, 0:64],
                                                   ar_sb[0:64, off:off + 64])
                                    nc.gpsimd.tensor_copy(kvbd[hp][64:P, 64:P],
                                                          ar_sb[64:P, off:off + 64])
                                for h in range(H):
                                    pr = (h % 2) * 64
                                    c = h // 2
                                    sc = 256 + c if c < 4 else 516 + (c - 4)
                                    nc.vector.tensor_copy(
                                        kd_sb[pr:pr + 64, h:h + 1],
                                        ar_sb[pr:pr + 64, sc:sc + 1])
                            for hp in hps:
                                attention(hp, ph4ps_d, ph4ps_z, ph4ps_a, ph4sb)

                # ===== Phase 5: output projection + residual + LayerNorm =====
                with (
                    tc.tile_pool(name="wop", bufs=1) as wop,
                    tc.tile_pool(name="ph5ps", bufs=3, space="PSUM") as ph5ps,
                    tc.tile_pool(name="ph5sb", bufs=3) as ph5sb,
                ):
                    wo_sb = wop.tile([P, KC, D], BF16)
                    for k in range(KC):
                        nc.sync.dma_start(wo_sb[:, k, :], r8(wo_d)[:, k, :])
                    for t in range(TC):
                        ts = slice(t * P, (t + 1) * P)
                        y = ph5sb.tile([P, D], F32, tag="y", name=f"y_{t}")
                        xr = ph5sb.tile([P, D], F32, tag="xr", name=f"xr_{t}")
                        nc.sync.dma_start(xr[:], xres.ap()[ts, :])
                        ops = ph5ps.tile([P, D], F32, tag="ops", name=f"ops_{t}")
                        for dh in range(2):
                            dsl = slice(dh * 512, (dh + 1) * 512)
                            for c in range(KC):
                                nc.tensor.matmul(ops[:, dsl], at_sb[:, c, ts], wo_sb[:, c, dsl],
                                                 start=(c == 0), stop=(c == KC - 1))
                        nc.vector.tensor_tensor(y[:], ops[:], xr[:], OP.add)
                        if apply_bias:
                            nc.vector.tensor_tensor(y[:], y[:], bo_b[:], OP.add)
                        stats = ph5sb.tile([P, 2, 6], F32, tag="stats", name=f"stats_{t}")
                        nc.vector.bn_stats(out=stats[:, 0, :], in_=y[:, :512])
                        nc.vector.bn_stats(out=stats[:, 1, :], in_=y[:, 512:])
                        mv = ph5sb.tile([P, 2], F32, tag="mv", name=f"mv_{t}")
                        nc.vector.bn_aggr(out=mv[:], in_=stats[:])
                        nc.scalar.activation(out=mv[:, 1:2], in_=mv[:, 1:2], func=AF.Sqrt,
                                             bias=eps_sb[:], scale=1.0)
                        nc.vector.reciprocal(mv[:, 1:2], mv[:, 1:2])
                        yo = ph5sb.tile([P, D], F32, tag="yo", name=f"yo_{t}")
                        nc.gpsimd.tensor_scalar(yo[:], y[:], mv[:, 0:1], mv[:, 1:2],
                                                OP.subtract, OP.mult)
                        if apply_gamma:
                            nc.vector.tensor_tensor(yo[:], yo[:], gamma_b[:], OP.mult)
                        if apply_beta:
                            nc.vector.tensor_tensor(yo[:], yo[:], beta_b[:], OP.add)
                        nc.sync.dma_start(out_d.ap()[ts, :], yo[:])

            wqp_cm.__exit__(None, None, None)

    nc.compile()
    return nc


def kernel(x, Wq, bq, Wk, bk, Wv, bv, Wo, bo, gamma, beta):
    global LAST_RESULT
    x = np.asarray(x, dtype=np.float32)
    f32 = np.float32
    bf16 = ml_dtypes.bfloat16

    apply_bias = any(np.any(np.asarray(b)) for b in (bq, bk, bv, bo))
    apply_gamma = not np.all(np.asarray(gamma) == 1.0)
    apply_beta = bool(np.any(np.asarray(beta)))

    nc = _build(apply_bias, apply_gamma, apply_beta)

    e4 = ml_dtypes.float8_e4m3

    def rn11(a):
        ai = np.ascontiguousarray(a, dtype=f32).view(np.uint32)
        keep = np.uint32(0xFFFFFFFF) << np.uint32(12)
        half = np.uint32(1) << np.uint32(11)
        return ((ai + half) & keep).view(f32)

    wq = np.asarray(Wq, f32)
    wk = np.asarray(Wk, f32)
    wv = np.asarray(Wv, f32)
    wo = np.asarray(Wo, f32).astype(bf16)
    wq1 = (wq * 2.0 ** 10).astype(e4)
    wq2 = ((wq - rn11(wq)) * 2.0 ** 16).astype(e4)
    e_sel = np.zeros((2, P), dtype=bf16)
    e_sel[0, :64] = 1
    e_sel[1, 64:] = 1

    in_maps = []
    for c in range(NCORES):
        b, half = c // 2, c % 2
        xs = x[b, half * TOK:(half + 1) * TOK]          # [2048, 1024]
        xst = np.ascontiguousarray(xs.T)
        m = {
            "xt": xst,
            "x1q": (xst * 2.0 ** 5).astype(e4),
            "x2q": ((xst - rn11(xst)) * 2.0 ** 11).astype(e4),
            "wq": wq, "wk": wk, "wq1": wq1, "wq2": wq2, "wv": wv, "wo": wo,
            "xres": np.ascontiguousarray(xs),
            "e_sel": e_sel,
        }
        if apply_bias:
            m.update(bq=np.asarray(bq, f32), bk=np.asarray(bk, f32),
                     bv=np.asarray(bv, f32), bo=np.asarray(bo, f32))
        if apply_gamma:
            m["gamma"] = np.asarray(gamma, f32)
        if apply_beta:
            m["beta"] = np.asarray(beta, f32)
        in_maps.append(m)

    import os
    try:
        LAST_RESULT = run_bass_kernel_spmd(nc, in_maps, core_ids=list(range(NCORES)))
    except ModuleNotFoundError:
        # no antenv.axon_hooks in this container -> NTFF tracing unavailable
        os.environ["BASS_NEVER_TRACE"] = "1"
        LAST_RESULT = run_bass_kernel_spmd(nc, in_maps, core_ids=list(range(NCORES)))
    out = np.empty((B, N, D), dtype=np.float32)
    for c in range(NCORES):
        b, half = c // 2, c % 2
        out[b, half * TOK:(half + 1) * TOK] = LAST_RESULT.results[c]["out"]
    return out


# revision 61
# speedup vs baseline: 1.1524x; 1.0097x over previous
"""Distributed Trainium2 kernel for the linear-attention transformer block.

Math (per batch element b):
  Q = elu(x @ Wq), K = elu(x @ Wk), V = x @ Wv   (per-head d=64)
  KV_h = K_h^T V_h  [64,64];  Ksum_h = sum_n K_h[n]  [64]
  attn_h = (Q_h @ KV_h) / (Q_h . Ksum_h)
  out = LayerNorm(x + attn @ Wo) * gamma + beta

Sharding: 16384 tokens over 8 cores (2048 each; core c owns batch c//2,
half c%2). Each core computes Q/K/V only for its tokens, partial KV/Ksum,
then a 266KB AllReduce over core pairs {2b, 2b+1} completes the KV stats;
attention + output projection + LayerNorm finish locally.

Precision: K/V/Q projections run in f32r (1 cycle/row on the PE for
moving dims >=256; round-to-nearest 11 mantissa bits). The 1/(Q.Ksum)
denominators pass near zero, so Q additionally gets an fp8-e4m3
DoubleRow correction pass (x_lo@Wq + x@Wq_lo at matched 2^21 scales,
half the cost of one f32r pass) and the denominator dot products run as
full-f32 matmuls on unrounded f32 Q/Ksum tiles. The attention numerator
(KV, aps, Wo) is bf16: its error is damped by the x + attn residual.
ELU is computed as max(exp(min(x,0)) - 1, x): ACT Relu(-x), ACT
Exp(-t), one fused DVE scalar_tensor_tensor.

Schedule: PE warmup fills the initial DMA window; phase 1 (K/V, KV,
Ksum) is PE-dense; the KV/Ksum AllReduce overlaps the first round of Q
projections. Phases 3/4 run as two rounds of (4 q_projs -> 4
attentions) so phase-3 PSUM pools can close before each attention
round (deep dps/zps/aps buffering) without deadlocking the 4-buffer qt
pool. Phase 5 (O-projection + LayerNorm) is PE-bound.
"""

import sys

sys.path.insert(0, "/opt/trn_rl_repo")

import numpy as np
import ml_dtypes

import concourse.bass as bass
import concourse.mybir as mybir
import concourse.tile as tile
from concourse import bacc
from concourse.bass_utils import run_bass_kernel_spmd

AF = mybir.ActivationFunctionType
OP = mybir.AluOpType
F32 = mybir.dt.float32
F32R = mybir.dt.float32r
BF16 = mybir.dt.bfloat16
FP8 = mybir.dt.float8e4   # IEEE e4m3: max finite 240 (NOT the fn variant)
DR = mybir.MatmulPerfMode.DoubleRow
QC_DESCALE = 2.0 ** -21   # x2q(2^11)@wq1(2^10) == x1q(2^5)@wq2(2^16)
V_DESCALE = 2.0 ** -15    # x1q(2^5) @ wv8(2^10)

B, N, D = 4, 4096, 1024
H, HD = 16, 64
TOK = 2048            # tokens per core
NCORES = 8
LN_EPS = 1e-3
P = 128
KC = D // P           # 8 contraction chunks
TC = TOK // P         # 16 token chunks of 128
TQ = TOK // 512       # 4 token chunks of 512

WARMUP_MM = 140   # covers the ~35us initial xt+wk DMA window
Q_CORR = True     # fp8 DoubleRow correction terms on the Q projection

LAST_RESULT = None    # BassKernelResults of the most recent run (for test.py)


def _build(apply_bias, apply_gamma, apply_beta):
    nc = bacc.Bacc("TRN2", target_bir_lowering=False, debug=False, num_devices=NCORES)

    xt = nc.dram_tensor("xt", [D, TOK], F32R, kind="ExternalInput")
    wq_d = nc.dram_tensor("wq", [D, D], F32R, kind="ExternalInput")
    wk_d = nc.dram_tensor("wk", [D, D], F32R, kind="ExternalInput")
    # fp8 operands: x1q = x*2^5, x2q = (x - rn11(x))*2^11, wq1 = Wq*2^10,
    # wq2 = (Wq - rn11(Wq))*2^16 (all e4m3, host-quantized).
    x1q_d = nc.dram_tensor("x1q", [D, TOK], FP8, kind="ExternalInput")
    x2q_d = nc.dram_tensor("x2q", [D, TOK], FP8, kind="ExternalInput")
    wq1_d = nc.dram_tensor("wq1", [D, D], FP8, kind="ExternalInput")
    wq2_d = nc.dram_tensor("wq2", [D, D], FP8, kind="ExternalInput")
    wv_d = nc.dram_tensor("wv", [D, D], F32R, kind="ExternalInput")
    wo_d = nc.dram_tensor("wo", [D, D], BF16, kind="ExternalInput")
    xres = nc.dram_tensor("xres", [TOK, D], F32, kind="ExternalInput")
    e_sel = nc.dram_tensor("e_sel", [2, P], BF16, kind="ExternalInput")
    if apply_bias:
        bq_d = nc.dram_tensor("bq", [D], F32, kind="ExternalInput")
        bk_d = nc.dram_tensor("bk", [D], F32, kind="ExternalInput")
        bv_d = nc.dram_tensor("bv", [D], F32, kind="ExternalInput")
        bo_d = nc.dram_tensor("bo", [D], F32, kind="ExternalInput")
    if apply_gamma:
        gamma_d = nc.dram_tensor("gamma", [D], F32, kind="ExternalInput")
    if apply_beta:
        beta_d = nc.dram_tensor("beta", [D], F32, kind="ExternalInput")
    out_d = nc.dram_tensor("out", [TOK, D], F32, kind="ExternalOutput")

    r8 = lambda t: t.ap().rearrange("(ko p) n -> p ko n", p=P)

    def bcast_row(dram_vec, sb_tile):
        # DMA-broadcast a [D] vector to [P, D] (stride-0 partition dim).
        src = bass.AP(
            tensor=dram_vec.ap().tensor,
            offset=dram_vec.ap().offset,
            ap=[[0, P]] + list(dram_vec.ap().ap),
        )
        nc.sync.dma_start(out=sb_tile, in_=src)

    with tile.TileContext(nc) as tc:
        with (
            tc.tile_pool(name="xpool", bufs=1) as xpool,
            tc.tile_pool(name="smalls", bufs=1) as smalls,
            tc.tile_pool(name="dram", bufs=1, space="DRAM") as dram,
        ):
            # ---- resident x^T (f32r) + fp8 variants ----
            xt_sb = xpool.tile([P, KC, TOK], F32R)
            x1q_sb = xpool.tile([P, KC, TOK], FP8)
            x2q_sb = xpool.tile([P, KC, TOK], FP8)

            e_sb = smalls.tile([2, P], BF16)
            nc.sync.dma_start(e_sb[:], e_sel.ap())
            ones_sb = smalls.tile([P, 1], F32)
            nc.vector.memset(ones_sb[:], 1.0)
            eps_sb = smalls.tile([P, 1], F32)
            nc.vector.memset(eps_sb[:], LN_EPS)
            # Per-head-pair block-diagonal KV operands (bf16, numerator only)
            # and Ksum columns (f32: the 1/(Q.Ksum) denominators cannot
            # afford f32r/bf16 operand rounding), filled after the AllReduce.
            kvbd = [smalls.tile([P, P], BF16, name=f"kvbd_{i}") for i in range(KC)]
            for kt in kvbd:
                nc.vector.memset(kt[:], 0.0)
            kd_sb = smalls.tile([P, H], F32)  # col h: Ksum_h at rows (h%2)*64
            nc.vector.memset(kd_sb[:], 0.0)
            acc = smalls.tile([P, 520], F32)
            nc.vector.memset(acc[:], 0.0)
            ar_sb = smalls.tile([P, 520], F32)
            if apply_bias:
                bq_sb = smalls.tile([P, KC], F32)   # per-partition layout for Q^T
                nc.sync.dma_start(bq_sb[:], bq_d.ap().rearrange("(ko p) -> p ko", p=P))
                bk_b = smalls.tile([P, D], F32)
                bv_b = smalls.tile([P, D], F32)
                bo_b = smalls.tile([P, D], F32)
                bcast_row(bk_d, bk_b[:])
                bcast_row(bv_d, bv_b[:])
                bcast_row(bo_d, bo_b[:])
            if apply_gamma:
                gamma_b = smalls.tile([P, D], F32)
                bcast_row(gamma_d, gamma_b[:])
            if apply_beta:
                beta_b = smalls.tile([P, D], F32)
                bcast_row(beta_d, beta_b[:])

            # PE warmup: matmuls on zeroed tiles hold the PE busy (p-state
            # ramp) while the first input DMAs are in flight.
            with (
                tc.tile_pool(name="warmsb", bufs=1) as warmsb,
                tc.tile_pool(name="warmps", bufs=2, space="PSUM") as warmps,
            ):
                warm_a = warmsb.tile([P, P], BF16)
                warm_b = warmsb.tile([P, 512], BF16)
                nc.gpsimd.memset(warm_a[:], 0.0)
                nc.gpsimd.memset(warm_b[:], 0.0)
                for w in range(WARMUP_MM):
                    wp = warmps.tile([P, 512], F32, tag="warm", name=f"warm_{w}")
                    nc.tensor.matmul(wp[:], warm_a[:], warm_b[:], start=True, stop=True)

            # Prefetch the first two Q-weight slices during phase 1.
            wqp_cm = tc.tile_pool(name="wqp", bufs=2)
            wqp = wqp_cm.__enter__()
            wq_tiles = {}

            def load_wq(hp):
                msl = slice(hp * P, (hp + 1) * P)
                wq_t = wqp.tile([P, KC, P], F32R, tag="wq", name=f"wq_{hp}")
                nc.sync.dma_start(wq_t[:], r8(wq_d)[:, :, msl])
                wq1_t = wqp.tile([P, KC, P], FP8, tag="wq1", name=f"wq1_{hp}")
                nc.sync.dma_start(wq1_t[:], r8(wq1_d)[:, :, msl])
                wq2_t = wqp.tile([P, KC, P], FP8, tag="wq2", name=f"wq2_{hp}")
                nc.sync.dma_start(wq2_t[:], r8(wq2_d)[:, :, msl])
                wq_tiles[hp] = (wq_t, wq1_t, wq2_t)

            # ================= Phase 1: K, V, partial KV + Ksum =================
            with (
                tc.tile_pool(name="wkv", bufs=1) as wkv,
                tc.tile_pool(name="kvps_pool", bufs=2, space="PSUM") as kvps_pool,
                tc.tile_pool(name="ph1ps", bufs=6, space="PSUM") as ph1ps,
                tc.tile_pool(name="ph1sb", bufs=3) as ph1sb,
            ):
                wk_sb = wkv.tile([P, KC, D], F32R)
                wv_sb = wkv.tile([P, KC, D], F32R)
                # The first K psum group needs ALL xt+wk chunks, so issue one
                # big DMA per tensor (per-instruction HWDGE/SEQ overhead is
                # the binding cost, not transfer): xt+wk first, then wv
                # (t=0's V loop), then the fp8 x copies (phase 3 only).
                for k in range(KC):
                    nc.sync.dma_start(xt_sb[:, k, :], r8(xt)[:, k, :])
                    nc.sync.dma_start(wk_sb[:, k, :], r8(wk_d)[:, k, :])
                for k in range(KC):
                    nc.sync.dma_start(wv_sb[:, k, :], r8(wv_d)[:, k, :])
                load_wq(0)
                load_wq(1)
                for k in range(KC):
                    nc.sync.dma_start(x1q_sb[:, k, :], r8(x1q_d)[:, k, :])
                    nc.sync.dma_start(x2q_sb[:, k, :], r8(x2q_d)[:, k, :])

                for t in range(TC):
                    ts = slice(t * P, (t + 1) * P)
                    kb_chunks = []
                    kvs_tiles = {}
                    for dh in range(2):
                        dsl = slice(dh * 512, (dh + 1) * 512)
                        kps = ph1ps.tile([P, 512], F32, tag="proj", name=f"kps_{t}_{dh}")
                        for k in range(KC):
                            nc.tensor.matmul(kps[:], xt_sb[:, k, ts], wk_sb[:, k, dsl],
                                             start=(k == 0), stop=(k == KC - 1))
                        if apply_bias:
                            kraw = ph1sb.tile([P, 512], F32, tag="kraw", name=f"kraw_{t}_{dh}")
                            nc.vector.tensor_tensor(kraw[:], kps[:], bk_b[:, dsl], OP.add)
                            ksrc = kraw
                        else:
                            ksrc = kps
                        # ELU = max(exp(min(x,0)) - 1, x); min(x,0) = -relu(-x)
                        kmin = ph1sb.tile([P, 512], F32, tag="kmin", name=f"kmin_{t}_{dh}")
                        nc.scalar.activation(kmin[:], ksrc[:], AF.Relu, scale=-1.0)
                        kexp = ph1sb.tile([P, 512], F32, tag="kexp", name=f"kexp_{t}_{dh}")
                        nc.scalar.activation(kexp[:], kmin[:], AF.Exp, scale=-1.0)
                        kf = ph1sb.tile([P, 512], F32, tag="kf", name=f"kf_{t}_{dh}")
                        nc.vector.scalar_tensor_tensor(kf[:], kexp[:], -1.0, ksrc[:],
                                                       OP.add, OP.max)
                        kb = ph1sb.tile([P, 512], BF16, tag="kb", name=f"kb_{t}_{dh}")
                        nc.gpsimd.tensor_copy(kb[:], kf[:])  # SBUF->SBUF cast
                        kb_chunks.append(kb)
                        # Ksum column blocks (f32 matmul against ones) go into
                        # cols [256, 260) of the shared kvs_t psum tile.
                        kvs_t = kvps_pool.tile([P, 260], F32, tag="kvs_t",
                                               name=f"kvs_t_{t}_{dh}")
                        kvs_tiles[dh] = kvs_t
                        for j in range(4):
                            nc.tensor.matmul(
                                kvs_t[:, 256 + j:257 + j], kf[:, j * P:(j + 1) * P],
                                ones_sb[:], start=True, stop=True, skip_group_check=True)
                    for dh in range(2):
                        dsl = slice(dh * 512, (dh + 1) * 512)
                        vps = ph1ps.tile([P, 512], F32, tag="proj", name=f"vps_{t}_{dh}")
                        for k in range(KC):
                            nc.tensor.matmul(vps[:], xt_sb[:, k, ts], wv_sb[:, k, dsl],
                                             start=(k == 0), stop=(k == KC - 1))
                        vb = ph1sb.tile([P, 512], BF16, tag="vb", name=f"vb_{t}_{dh}")
                        if apply_bias:
                            nc.vector.tensor_tensor(vb[:], vps[:], bv_b[:, dsl], OP.add)
                        else:
                            nc.vector.tensor_copy(vb[:], vps[:])
                        kb = kb_chunks[dh]
                        kvs_t = kvs_tiles[dh]
                        for hh in range(8):
                            h = dh * 8 + hh
                            pr = (h % 2) * 64
                            fc = (h // 2) * 64 - dh * 256
                            nc.tensor.matmul(
                                kvs_t[pr:pr + 64, fc:fc + 64],
                                kb[:, hh * 64:(hh + 1) * 64],
                                vb[:, hh * 64:(hh + 1) * 64],
                                start=True, stop=True,
                                tile_position=(0, pr), skip_group_check=True)
                        nc.vector.tensor_tensor(
                            acc[:, dh * 260:(dh + 1) * 260],
                            acc[:, dh * 260:(dh + 1) * 260], kvs_t[:], OP.add)

            # ========== Phases 2-4: AllReduce; Q^T; attention ==========
            with tc.tile_pool(name="late", bufs=1) as late:
                at_sb = late.tile([P, KC, TOK], BF16)

                # -- AllReduce of the packed KV/Ksum accumulator --
                cc_in = dram.tile([P, 520], F32)
                cc_out = dram.tile([P, 520], F32)
                nc.sync.dma_start(cc_in[:], acc[:])
                nc.gpsimd.collective_compute(
                    "AllReduce", OP.add,
                    replica_groups=[[0, 1], [2, 3], [4, 5], [6, 7]],
                    ins=[cc_in[:].opt()], outs=[cc_out[:].opt()])
                nc.sync.dma_start(ar_sb[:], cc_out[:])

                with tc.tile_pool(name="qtp", bufs=4) as qtp:
                    qt_tiles = {}

                    def q_proj(hp, ph3ps, ph3psc, ph3sb):
                        wq_t, wq1_t, wq2_t = wq_tiles.pop(hp)
                        qt = qtp.tile([P, TOK], F32, tag="qt", name=f"qt_{hp}")
                        qt_tiles[hp] = qt
                        for tq in range(TQ):
                            tsl = slice(tq * 512, (tq + 1) * 512)
                            qps = ph3ps.tile([P, 512], F32, tag="qps", name=f"qps_{hp}_{tq}")
                            for k in range(KC):
                                nc.tensor.matmul(qps[:], wq_t[:, k, :], xt_sb[:, k, tsl],
                                                 start=(k == 0), stop=(k == KC - 1))
                            qsum = ph3sb.tile([P, 512], F32, tag="qsum", name=f"qsum_{hp}_{tq}")
                            if Q_CORR:
                                # fp8 DoubleRow correction: 2^22*(x_lo@Wq + x@Wq_lo)
                                cps = ph3psc.tile([P, 512], F32, tag="cps", name=f"cps_{hp}_{tq}")
                                for k2 in range(KC // 2):
                                    nc.tensor.matmul(cps[:], wq1_t[:, 2 * k2:2 * k2 + 2, :],
                                                     x2q_sb[:, 2 * k2:2 * k2 + 2, tsl],
                                                     start=(k2 == 0), stop=False, perf_mode=DR)
                                for k2 in range(KC // 2):
                                    nc.tensor.matmul(cps[:], wq2_t[:, 2 * k2:2 * k2 + 2, :],
                                                     x1q_sb[:, 2 * k2:2 * k2 + 2, tsl],
                                                     start=False, stop=(k2 == KC // 2 - 1),
                                                     perf_mode=DR)
                                cc = ph3sb.tile([P, 512], F32, tag="cc", name=f"cc_{hp}_{tq}")
                                nc.scalar.activation(cc[:], cps[:], AF.Copy)
                                nc.vector.scalar_tensor_tensor(qsum[:], cc[:], QC_DESCALE,
                                                               qps[:], OP.mult, OP.add)
                            else:
                                nc.vector.tensor_copy(qsum[:], qps[:])
                            if apply_bias:
                                nc.vector.tensor_scalar(qsum[:], qsum[:], bq_sb[:, hp:hp + 1],
                                                        None, OP.add)
                            qmin = ph3sb.tile([P, 512], F32, tag="qmin", name=f"qmin_{hp}_{tq}")
                            nc.scalar.activation(qmin[:], qsum[:], AF.Relu, scale=-1.0)
                            qexp = ph3sb.tile([P, 512], F32, tag="qexp", name=f"qexp_{hp}_{tq}")
                            nc.scalar.activation(qexp[:], qmin[:], AF.Exp, scale=-1.0)
                            nc.vector.scalar_tensor_tensor(qt[:, tsl], qexp[:], -1.0,
                                                           qsum[:], OP.add, OP.max)

                    def attention(hp, ph4ps_d, ph4ps_z, ph4ps_a, ph4sb):
                        qt = qt_tiles.pop(hp)
                        qbs = []
                        for tq in range(TQ):
                            # qb casts are off the dps->at critical chain;
                            # emit first so Pool runs ahead.
                            qb = ph4sb.tile([P, 512], BF16, tag="qb",
                                            name=f"qb_{hp}_{tq}")
                            nc.gpsimd.tensor_copy(qb[:], qt[:, tq * 512:(tq + 1) * 512])
                            qbs.append(qb)
                        for tq in range(TQ):
                            tsl = slice(tq * 512, (tq + 1) * 512)
                            dps = ph4ps_d.tile([2, 512], F32, tag="dps", name=f"dps_{hp}_{tq}")
                            nc.tensor.matmul(dps[:], kd_sb[:, 2 * hp:2 * hp + 2],
                                             qt[:, tsl], start=True, stop=True)
                            zr = ph4sb.tile([2, 512], BF16, tag="zr", name=f"zr_{hp}_{tq}")
                            with nc.allow_low_precision(reason="Z is a per-token scale"):
                                nc.vector.reciprocal(zr[:], dps[:])
                            zps = ph4ps_z.tile([P, 512], F32, tag="zps", name=f"zps_{hp}_{tq}")
                            nc.tensor.matmul(zps[:], e_sb[:], zr[:], start=True, stop=True)
                            zf = ph4sb.tile([P, 512], BF16, tag="zf", name=f"zf_{hp}_{tq}")
                            nc.scalar.activation(zf[:], zps[:], AF.Copy)
                            aps = ph4ps_a.tile([P, 512], F32, tag="aps", name=f"aps_{hp}_{tq}")
                            nc.tensor.matmul(aps[:], kvbd[hp][:], qbs[tq][:],
                                             start=True, stop=True)
                            nc.vector.tensor_tensor(at_sb[:, hp, tsl], aps[:], zf[:], OP.mult)

                    # Two rounds of (4 q_projs -> 4 attentions). Within each
                    # round the phase-3 PSUM pools close before attention so
                    # the dps/zps/aps chain gets deep buffering; the round
                    # split keeps the qt pool (4 bufs) cycle-free.
                    for rnd in range(2):
                        hps = range(4 * rnd, 4 * rnd + 4)
                        with (
                            tc.tile_pool(name=f"ph3ps_{rnd}", bufs=3, space="PSUM") as ph3ps,
                            tc.tile_pool(name=f"ph3psc_{rnd}", bufs=1, space="PSUM") as ph3psc,
                            tc.tile_pool(name=f"ph3sb_{rnd}", bufs=2) as ph3sb,
                        ):
                            for hp in hps:
                                if 2 <= hp + 1 < KC:
                                    load_wq(hp + 1)
                                q_proj(hp, ph3ps, ph3psc, ph3sb)
                        with (
                            tc.tile_pool(name=f"ph4ps_d_{rnd}", bufs=3, space="PSUM") as ph4ps_d,
                            tc.tile_pool(name=f"ph4ps_z_{rnd}", bufs=2, space="PSUM") as ph4ps_z,
                            tc.tile_pool(name=f"ph4ps_a_{rnd}", bufs=3, space="PSUM") as ph4ps_a,
                            tc.tile_pool(name=f"ph4sb_{rnd}", bufs=4) as ph4sb,
                        ):
                            if rnd == 0:
                                # Unpack AllReduce result into KV/Ksum
                                # operands, spread across engines.
                                for hp in range(KC):
                                    off = (hp // 4) * 260 + (hp % 4) * 64
                                    nc.scalar.copy(kvbd[hp][0:64, 0:64],
                                                   ar_sb[0:64, off:off + 64])
                                    nc.gpsimd.tensor_copy(kvbd[hp][64:P, 64:P],
                                                          ar_sb[64:P, off:off + 64])
                                for h in range(H):
                                    pr = (h % 2) * 64
                                    c = h // 2
                                    sc = 256 + c if c < 4 else 516 + (c - 4)
                                    nc.vector.tensor_copy(
                                        kd_sb[pr:pr + 64, h:h + 1],
                                        ar_sb[pr:pr + 64, sc:sc + 1])
                            for hp in hps:
                                attention(hp, ph4ps_d, ph4ps_z, ph4ps_a, ph4sb)

                # ===== Phase 5: output projection + residual + LayerNorm =====
                with (
                    tc.tile_pool(name="wop", bufs=1) as wop,
                    tc.tile_pool(name="ph5ps", bufs=3, space="PSUM") as ph5ps,
                    tc.tile_pool(name="ph5sb", bufs=3) as ph5sb,
                ):
                    wo_sb = wop.tile([P, KC, D], BF16)
                    for k in range(KC):
                        nc.sync.dma_start(wo_sb[:, k, :], r8(wo_d)[:, k, :])
                    for t in range(TC):
                        ts = slice(t * P, (t + 1) * P)
                        y = ph5sb.tile([P, D], F32, tag="y", name=f"y_{t}")
                        xr = ph5sb.tile([P, D], F32, tag="xr", name=f"xr_{t}")
                        nc.sync.dma_start(xr[:], xres.ap()[ts, :])
                        ops = ph5ps.tile([P, D], F32, tag="ops", name=f"ops_{t}")
                        for dh in range(2):
                            dsl = slice(dh * 512, (dh + 1) * 512)
                            for c in range(KC):
                                nc.tensor.matmul(ops[:, dsl], at_sb[:, c, ts], wo_sb[:, c, dsl],
                                                 start=(c == 0), stop=(c == KC - 1))
                        nc.vector.tensor_tensor(y[:], ops[:], xr[:], OP.add)
                        if apply_bias:
                            nc.vector.tensor_tensor(y[:], y[:], bo_b[:], OP.add)
                        stats = ph5sb.tile([P, 2, 6], F32, tag="stats", name=f"stats_{t}")
                        nc.vector.bn_stats(out=stats[:, 0, :], in_=y[:, :512])
                        nc.vector.bn_stats(out=stats[:, 1, :], in_=y[:, 512:])
                        mv = ph5sb.tile([P, 2], F32, tag="mv", name=f"mv_{t}")
                        nc.vector.bn_aggr(out=mv[:], in_=stats[:])
                        nc.scalar.activation(out=mv[:, 1:2], in_=mv[:, 1:2], func=AF.Sqrt,
                                             bias=eps_sb[:], scale=1.0)
                        nc.vector.reciprocal(mv[:, 1:2], mv[:, 1:2])
                        yo = ph5sb.tile([P, D], F32, tag="yo", name=f"yo_{t}")
                        nc.gpsimd.tensor_scalar(yo[:], y[:], mv[:, 0:1], mv[:, 1:2],
                                                OP.subtract, OP.mult)
                        if apply_gamma:
                            nc.vector.tensor_tensor(yo[:], yo[:], gamma_b[:], OP.mult)
                        if apply_beta:
                            nc.vector.tensor_tensor(yo[:], yo[:], beta_b[:], OP.add)
                        nc.sync.dma_start(out_d.ap()[ts, :], yo[:])

            wqp_cm.__exit__(None, None, None)

    nc.compile()
    return nc


def kernel(x, Wq, bq, Wk, bk, Wv, bv, Wo, bo, gamma, beta):
    global LAST_RESULT
    x = np.asarray(x, dtype=np.float32)
    f32 = np.float32
    bf16 = ml_dtypes.bfloat16

    apply_bias = any(np.any(np.asarray(b)) for b in (bq, bk, bv, bo))
    apply_gamma = not np.all(np.asarray(gamma) == 1.0)
    apply_beta = bool(np.any(np.asarray(beta)))

    nc = _build(apply_bias, apply_gamma, apply_beta)

    e4 = ml_dtypes.float8_e4m3

    def rn11(a):
        ai = np.ascontiguousarray(a, dtype=f32).view(np.uint32)
        keep = np.uint32(0xFFFFFFFF) << np.uint32(12)
        half = np.uint32(1) << np.uint32(11)
        return ((ai + half) & keep).view(f32)

    wq = np.asarray(Wq, f32)
    wk = np.asarray(Wk, f32)
    wv = np.asarray(Wv, f32)
    wo = np.asarray(Wo, f32).astype(bf16)
    wq1 = (wq * 2.0 ** 10).astype(e4)
    wq2 = ((wq - rn11(wq)) * 2.0 ** 16).astype(e4)
    e_sel = np.zeros((2, P), dtype=bf16)
    e_sel[0, :64] = 1
    e_sel[1, 64:] = 1

    in_maps = []
    for c in range(NCORES):
        b, half = c // 2, c % 2
        xs = x[b, half * TOK:(half + 1) * TOK]          # [2048, 1024]
        xst = np.ascontiguousarray(xs.T)
        m = {
            "xt": xst,
            "x1q": (xst * 2.0 ** 5).astype(e4),
            "x2q": ((xst - rn11(xst)) * 2.0 ** 11).astype(e4),
            "wq": wq, "wk": wk, "wq1": wq1, "wq2": wq2, "wv": wv, "wo": wo,
            "xres": np.ascontiguousarray(xs),
            "e_sel": e_sel,
        }
        if apply_bias:
            m.update(bq=np.asarray(bq, f32), bk=np.asarray(bk, f32),
                     bv=np.asarray(bv, f32), bo=np.asarray(bo, f32))
        if apply_gamma:
            m["gamma"] = np.asarray(gamma, f32)
        if apply_beta:
            m["beta"] = np.asarray(beta, f32)
        in_maps.append(m)

    import os
    try:
        LAST_RESULT = run_bass_kernel_spmd(nc, in_maps, core_ids=list(range(NCORES)))
    except ModuleNotFoundError:
        # no antenv.axon_hooks in this container -> NTFF tracing unavailable
        os.environ["BASS_NEVER_TRACE"] = "1"
        LAST_RESULT = run_bass_kernel_spmd(nc, in_maps, core_ids=list(range(NCORES)))
    out = np.empty((B, N, D), dtype=np.float32)
    for c in range(NCORES):
        b, half = c // 2, c % 2
        out[b, half * TOK:(half + 1) * TOK] = LAST_RESULT.results[c]["out"]
    return out
